# revision 23
# baseline (speedup 1.0000x reference)
"""MultiHeadAttention forward on 8 Trainium2 NeuronCores.

Tensor-parallel over heads: each core owns 2 of 16 heads (d_loc=256 of the
2048 QKV output columns, and the matching 256 rows of Wo). Each core
computes a full-shape partial output; the host sums the 8 partials and
adds bo (+ the folded bv @ Wo term).

Problem shape: x [2, 2048, 2048], 16 heads, d_k = 128, fp32 in/out.

Design (measured 385us vs the 530us fp32r baseline; PE busy ~91%, rel err
~5e-3 vs the 2e-2 gate):
  - All matmul operands in bf16 (PE still streams 1 row/cycle, but DMA and
    SBUF halve -> faster startup, FWL weight loads, smaller output tail).
    PSUM accumulation stays fp32.
  - Softmax denominator: exp tiles are summed elementwise by THREE chains
    (two on DVE, one on the otherwise-idle GPSIMD, each starting with a
    2-input add) and reduced over partitions by 3 accumulated ones-matmuls
    per (h,chunk) instead of 16 -> frees ~107k PE cycles (~45us) while
    keeping DVE under the PE rate.
  - qT/kT/avT/v tiles double-buffered across batches so batch 1's
    projections overlap batch 0's attention (kills the mid-kernel stall).
  - Batch 1's projection runs QK for all chunks FIRST, then V (x re-read
    from HBM): batch 1's attention scores+exps then overlap its own V pass,
    hiding part of the ~90us of ACT exp work (the softest engine bound).
  - PSUM (8 banks): proj 2 (QK+V groups + batch-1 O-proj), scores 3
    (+ batch-0 denominators), AV 2, batch-0 O-proj 1 (trickles under
    attention); batch-1 denominators use the freed O bank; the last chunk's
    O-proj also borrows the freed score banks to shorten the tail.
  - Output projection emitted per chunk for both batches; y written bf16 in
    half-row DMAs so the final drain is ~1MB; first x/w tile pairs go out
    on the scalar/gpsimd DMA queues to beat the sync-queue spin-up.
"""

import functools
from contextlib import ExitStack

import numpy as np

D_MODEL = 2048
NUM_HEADS = 16
DK = 128
B = 2
T = 2048
BT = B * T
N_CORES = 8
H_LOC = NUM_HEADS // N_CORES  # 2 heads per core
D_LOC = H_LOC * DK  # 256
C_TILES = D_MODEL // 128  # 16
TQ = 512  # tq chunk width (one PSUM bank in fp32)
NCH = T // TQ  # 4 chunks per batch
TK_TILES = T // 128  # 16


def _body(ctx, tc, xT, wqkv, bqk, wo, y):
    import concourse.bass as bass  # noqa: F401
    from concourse import mybir

    nc = tc.nc
    f32 = mybir.dt.float32
    bf16 = mybir.dt.bfloat16
    Exp = mybir.ActivationFunctionType.Exp
    inv_sqrt_dk = 1.0 / float(np.sqrt(DK))

    # ---------------- resident tensors ----------------
    # Interleave the first x-chunk's tile loads with the weight loads so the
    # first projection matmuls can start after ~2 DMAs instead of after the
    # whole weight preload.
    wpool = ctx.enter_context(tc.tile_pool(name="wpool", bufs=1))
    x_pool = ctx.enter_context(tc.tile_pool(name="x_pool", bufs=48))

    w_tiles = []
    xt_pre = []
    for i in range(C_TILES):
        # first pairs go on otherwise-idle engine DMA queues so tile 0 lands
        # in ~1us instead of waiting behind the whole wave on sync's queues
        eng = {1: nc.scalar, 2: nc.gpsimd}.get(i, nc.sync)
        xti = x_pool.tile([128, TQ], bf16, tag="xt", name=f"xtpre{i}")
        eng.dma_start(out=xti, in_=xT[i * 128 : (i + 1) * 128, 0:TQ])
        xt_pre.append(xti)
        wt = wpool.tile([128, 3 * D_LOC], bf16, tag=f"w{i}", name=f"w{i}")
        eng.dma_start(out=wt, in_=wqkv[i * 128 : (i + 1) * 128, :])
        w_tiles.append(wt)
    bqk_sb = wpool.tile([128, 4], f32, tag="bqk", name="bqk")
    nc.sync.dma_start(out=bqk_sb, in_=bqk[:, :])

    wo_tiles = []
    for d in range(H_LOC):
        wot = wpool.tile([128, D_MODEL], bf16, tag=f"wo{d}", name=f"wo{d}")
        nc.sync.dma_start(out=wot, in_=wo[d * 128 : (d + 1) * 128, :])
        wo_tiles.append(wot)

    ones = wpool.tile([128, 128], bf16, tag="ones", name="ones")
    nc.vector.memset(ones, 1.0)

    # ---------------- pools ----------------
    qkv_pool = ctx.enter_context(tc.tile_pool(name="qkv_pool", bufs=2))
    av_pool = ctx.enter_context(tc.tile_pool(name="av_pool", bufs=2))
    es_pool = ctx.enter_context(tc.tile_pool(name="es_pool", bufs=8))
    acc_pool = ctx.enter_context(tc.tile_pool(name="acc_pool", bufs=6))
    rc_pool = ctx.enter_context(tc.tile_pool(name="rc_pool", bufs=2))
    y_pool = ctx.enter_context(tc.tile_pool(name="y_pool", bufs=3))

    # PSUM: 8 banks total.
    #   ps_p  x2: QK-proj + V-proj groups (+ batch-1's O-proj psums — proj done)
    #   ps_s  x3: score tiles + the per-unit denominator psum
    #   ps_av x2: AV accumulators (double-buffered across units)
    #   ps_o  x1: batch-0's O-proj psums (trickle, absorbed by attention gaps)
    ps_p = ctx.enter_context(tc.tile_pool(name="ps_p", bufs=2, space="PSUM"))
    ps_s = ctx.enter_context(tc.tile_pool(name="ps_s", bufs=3, space="PSUM"))
    ps_av = ctx.enter_context(tc.tile_pool(name="ps_av", bufs=2, space="PSUM"))
    ps_o = ctx.enter_context(tc.tile_pool(name="ps_o", bufs=1, space="PSUM"))



    for b in range(B):
        # ---------------- phase P: QKV projections ----------------
        qT = [
            qkv_pool.tile([128, T], bf16, tag=f"qT{d}", name=f"qT{d}_{b}")
            for d in range(H_LOC)
        ]
        kT = [
            qkv_pool.tile([128, T], bf16, tag=f"kT{d}", name=f"kT{d}_{b}")
            for d in range(H_LOC)
        ]
        v_t = [
            qkv_pool.tile([128, D_LOC], bf16, tag=f"v{t}", name=f"v{t}_{b}")
            for t in range(TK_TILES)
        ]

        def load_x_chunk(ch, pass_name):
            t0 = b * T + ch * TQ
            xt = []
            for i in range(C_TILES):
                xti = x_pool.tile(
                    [128, TQ], bf16, tag="xt", name=f"xt{pass_name}{b}_{ch}_{i}"
                )
                nc.sync.dma_start(
                    out=xti, in_=xT[i * 128 : (i + 1) * 128, t0 : t0 + TQ]
                )
                xt.append(xti)
            return xt

        def emit_qk(ch, xt):
            # QT / KT: j -> (qT, kT)[j // 2][j % 2]
            for j, dest in enumerate((qT[0], qT[1], kT[0], kT[1])):
                # chunk 0 only: borrow the (idle until attention) o/av banks so
                # four projection groups can overlap the initial x/w DMAs.
                if b == 0 and ch == 0 and j == 2:
                    ps = ps_o.tile([128, TQ], f32, tag="o", name=f"psqk02_{j}")
                elif b == 0 and ch == 0 and j == 3:
                    ps = ps_av.tile([128, TQ], f32, tag="av", name=f"psqk03_{j}")
                else:
                    ps = ps_p.tile(
                        [128, TQ], f32, tag="p", name=f"psqk{b}_{ch}_{j}"
                    )
                for i in range(C_TILES):
                    nc.tensor.matmul(
                        ps,
                        w_tiles[i][:, j * 128 : (j + 1) * 128],
                        xt[i],
                        start=(i == 0),
                        stop=(i == C_TILES - 1),
                    )
                # PSUM -> SBUF with per-partition bias add
                nc.vector.tensor_scalar_add(
                    dest[:, ch * TQ : (ch + 1) * TQ], ps, bqk_sb[:, j : j + 1]
                )

        def emit_v(ch, xt):
            # V: natural [t, d] layout (bv folded host-side as bv @ Wo)
            for ts in range(TQ // 128):
                t_idx = ch * (TQ // 128) + ts
                ps = ps_p.tile([128, TQ], f32, tag="p", name=f"psv{b}_{t_idx}")
                psv = ps[:, :D_LOC]
                for i in range(C_TILES):
                    nc.tensor.matmul(
                        psv,
                        xt[i][:, ts * 128 : (ts + 1) * 128],
                        w_tiles[i][:, 2 * D_LOC : 3 * D_LOC],
                        start=(i == 0),
                        stop=(i == C_TILES - 1),
                    )
                nc.vector.tensor_copy(v_t[t_idx], psv)

        if b == 0:
            # interleaved QK+V per chunk; batch-1's projection fills this
            # batch's attention gaps later.
            for ch in range(NCH):
                xt = xt_pre if ch == 0 else load_x_chunk(ch, "qv")
                emit_qk(ch, xt)
                emit_v(ch, xt)
        else:
            # QK for all chunks first, then V (x re-read from HBM): the last
            # batch's attention scores+exps can then overlap its own V pass,
            # hiding part of the ~90us of ACT exp work behind V matmuls.
            for ch in range(NCH):
                emit_qk(ch, load_x_chunk(ch, "q"))
            for ch in range(NCH):
                emit_v(ch, load_x_chunk(ch, "v"))

        # ---------------- phase A: attention + output projection ----------
        avT = [
            av_pool.tile([128, T], bf16, tag=f"avT{d}", name=f"avT{d}_{b}")
            for d in range(H_LOC)
        ]

        # denominator chains: c on GPSIMD (SBUF-only; idle engine) gets early
        # tiles so its slower ops finish before the pdn matmuls; a,b on DVE.
        # Each chain starts with a 2-input add (no init copy needed).
        CHAINS = {  # tk -> (chain index, peer tk for the starting add)
            0: (2, None), 2: (2, 0), 4: (2, None), 6: (2, None),
            8: (2, None), 10: (2, None),
            1: (0, None), 5: (0, 1), 9: (0, None), 12: (0, None), 14: (0, None),
            3: (1, None), 7: (1, 3), 11: (1, None), 13: (1, None), 15: (1, None),
        }

        for ch in range(NCH):
            for h in range(H_LOC):
                pav = ps_av.tile([128, TQ], f32, tag="av", name=f"pav{b}_{h}_{ch}")
                accs = [
                    acc_pool.tile(
                        [128, TQ], bf16, tag="acc", name=f"acc{cn}{b}_{h}_{ch}"
                    )
                    for cn in "abc"
                ]
                es_tiles = {}
                for tk in range(TK_TILES):
                    pss = ps_s.tile(
                        [128, TQ], f32, tag="s", name=f"pss{b}_{h}_{ch}_{tk}"
                    )
                    nc.tensor.matmul(
                        pss,
                        kT[h][:, tk * 128 : (tk + 1) * 128],
                        qT[h][:, ch * TQ : (ch + 1) * TQ],
                        start=True,
                        stop=True,
                    )
                    es = es_pool.tile(
                        [128, TQ], bf16, tag="es", name=f"es{b}_{h}_{ch}_{tk}"
                    )
                    nc.scalar.activation(es, pss, Exp, scale=inv_sqrt_dk)
                    es_tiles[tk] = es
                    ci, peer = CHAINS[tk]
                    eng = nc.gpsimd if ci == 2 else nc.vector
                    if peer is not None:
                        eng.tensor_add(accs[ci], es_tiles[peer], es)
                    elif peer is None and tk not in (0, 1, 3):
                        eng.tensor_add(accs[ci], accs[ci], es)
                    nc.tensor.matmul(
                        pav,
                        v_t[tk][:, h * 128 : (h + 1) * 128],
                        es,
                        start=(tk == 0),
                        stop=(tk == TK_TILES - 1),
                    )
                # join the three chains on DVE (it has slack), then ONE
                # partition-sum matmul — spends 2 DVE adds to save 2 PE MMs.
                nc.vector.tensor_add(accs[0], accs[0], accs[1])
                nc.vector.tensor_add(accs[0], accs[0], accs[2])
                # b=1: the o-bank is free (no b1 O-proj there) — using it keeps
                # the next unit's score psums off the recip's rotation chain.
                if b == 1:
                    pdn = ps_o.tile([128, TQ], f32, tag="o", name=f"pdn{b}_{h}_{ch}")
                else:
                    pdn = ps_s.tile([128, TQ], f32, tag="s", name=f"pdn{b}_{h}_{ch}")
                nc.tensor.matmul(pdn, ones, accs[0], start=True, stop=True)
                rc = rc_pool.tile([128, TQ], f32, tag="rc", name=f"rc{b}_{h}_{ch}")
                nc.vector.reciprocal_approx_fast(out=rc, in_=pdn)
                nc.vector.tensor_mul(avT[h][:, ch * TQ : (ch + 1) * TQ], pav, rc)

            # output projection for the t-tiles whose avT chunk just finished.
            # b=0: the single o-bank trickles under attention/proj-b1 matmuls;
            # b=1: proj is done, so the p-pool's 2 banks pipeline; the last
            # chunk also borrows the (finished) s-pool to shorten the tail.
            last_chunk = b == B - 1 and ch == NCH - 1
            for t in range(ch * (TQ // 128), (ch + 1) * (TQ // 128)):
                row0 = b * T + t * 128
                ystage = y_pool.tile(
                    [128, D_MODEL], bf16, tag="y", name=f"ys{b}_{t}"
                )
                for nch in range(NCH):
                    if b == 0:
                        pso = ps_o.tile(
                            [128, TQ], f32, tag="o", name=f"pso{b}_{t}_{nch}"
                        )
                    elif last_chunk and nch % 2 == 1:
                        pso = ps_s.tile(
                            [128, TQ], f32, tag="s", name=f"pso{b}_{t}_{nch}"
                        )
                    else:
                        pso = ps_p.tile(
                            [128, TQ], f32, tag="p", name=f"pso{b}_{t}_{nch}"
                        )
                    for d in range(H_LOC):
                        nc.tensor.matmul(
                            pso,
                            avT[d][:, t * 128 : (t + 1) * 128],
                            wo_tiles[d][:, nch * TQ : (nch + 1) * TQ],
                            start=(d == 0),
                            stop=(d == H_LOC - 1),
                        )
                    # b=0: one copy in four goes to the scalar engine; b=1 the
                    # scalar engine is exp-bound, so everything stays on DVE.
                    dst = ystage[:, nch * TQ : (nch + 1) * TQ]
                    if nch == 3 and b == 0:
                        nc.scalar.copy(dst, pso)
                    else:
                        nc.vector.tensor_copy(dst, pso)
                    # write out each half as soon as its two copies land, so
                    # the final drain is ~1MB, not the whole chunk
                    if last_chunk:
                        # quarter-granularity writes: the final drain starts
                        # as early as possible after each copy
                        nc.sync.dma_start(
                            out=y[row0 : row0 + 128, nch * TQ : (nch + 1) * TQ],
                            in_=dst,
                        )
                    elif nch == 1:
                        nc.sync.dma_start(
                            out=y[row0 : row0 + 128, : D_MODEL // 2],
                            in_=ystage[:, : D_MODEL // 2],
                        )
                    elif nch == 3:
                        nc.sync.dma_start(
                            out=y[row0 : row0 + 128, D_MODEL // 2 :],
                            in_=ystage[:, D_MODEL // 2 :],
                        )


@functools.cache
def _build():
    from concourse import bacc
    import concourse.tile as tile
    from concourse import mybir

    nc = bacc.Bacc(
        "TRN2",
        target_bir_lowering=False,
        debug=False,
        enable_asserts=False,
        num_devices=N_CORES,
    )
    f32 = mybir.dt.float32
    bf16 = mybir.dt.bfloat16
    xT = nc.dram_tensor("xT", [D_MODEL, BT], bf16, kind="ExternalInput").ap()
    wqkv = nc.dram_tensor(
        "wqkv", [D_MODEL, 3 * D_LOC], bf16, kind="ExternalInput"
    ).ap()
    bqk = nc.dram_tensor("bqk", [128, 4], f32, kind="ExternalInput").ap()
    wo = nc.dram_tensor("wo", [D_LOC, D_MODEL], bf16, kind="ExternalInput").ap()
    y = nc.dram_tensor("y", [BT, D_MODEL], bf16, kind="ExternalOutput").ap()

    with tile.TileContext(nc) as tc:
        with ExitStack() as ctx:
            _body(ctx, tc, xT, wqkv, bqk, wo, y)
    nc.compile()
    return nc


def _shard_inputs(x, Wq, bq, Wk, bk, Wv, bv, Wo, bo):
    """Host-side sharding: returns per-core input maps (bf16 operands)."""
    import ml_dtypes

    f = np.float32
    b16 = ml_dtypes.bfloat16
    xT = np.ascontiguousarray(
        np.asarray(x, f).reshape(BT, D_MODEL).T.astype(b16)
    )
    Wq, Wk, Wv, Wo = (np.asarray(a, f) for a in (Wq, Wk, Wv, Wo))
    bq, bk, bv = (np.asarray(a, f) for a in (bq, bk, bv))
    in_maps = []
    for c in range(N_CORES):
        sl = slice(c * D_LOC, (c + 1) * D_LOC)
        wqkv_pad = np.ascontiguousarray(
            np.concatenate([Wq[:, sl], Wk[:, sl], Wv[:, sl]], axis=1).astype(b16)
        )
        bqk_t = np.ascontiguousarray(
            np.stack(
                [
                    bq[sl][:128],
                    bq[sl][128:],
                    bk[sl][:128],
                    bk[sl][128:],
                ],
                axis=1,
            )
        )
        wo_loc = np.ascontiguousarray(Wo[sl, :].astype(b16))
        in_maps.append({"xT": xT, "wqkv": wqkv_pad, "bqk": bqk_t, "wo": wo_loc})
    return in_maps


def _run(in_maps, trace=False, **kwargs):
    from concourse.bass_utils import run_bass_kernel_spmd

    nc = _build()
    return run_bass_kernel_spmd(
        nc, in_maps, core_ids=list(range(N_CORES)), trace=trace, **kwargs
    )


def kernel(x, Wq, bq, Wk, bk, Wv, bv, Wo, bo):
    in_maps = _shard_inputs(x, Wq, bq, Wk, bk, Wv, bv, Wo, bo)
    res = _run(in_maps, trace=False)
    acc = np.zeros((BT, D_MODEL), np.float32)
    for rmap in res.results:
        acc += np.asarray(rmap["y"]).astype(np.float32)
    acc += np.asarray(bo, np.float32)[None, :]
    acc += (np.asarray(bv, np.float32) @ np.asarray(Wo, np.float32))[None, :]
    return acc.reshape(B, T, D_MODEL)


# revision 24
# speedup vs baseline: 1.0109x; 1.0109x over previous
"""MultiHeadAttention forward on 8 Trainium2 NeuronCores.

Tensor-parallel over heads: each core owns 2 of 16 heads (d_loc=256 of the
2048 QKV output columns, and the matching 256 rows of Wo). Each core
computes a full-shape partial output; the host sums the 8 partials and
adds bo (+ the folded bv @ Wo term).

Problem shape: x [2, 2048, 2048], 16 heads, d_k = 128, fp32 in/out.

Design (measured 385us vs the 530us fp32r baseline; PE busy ~91%, rel err
~5e-3 vs the 2e-2 gate):
  - All matmul operands in bf16 (PE still streams 1 row/cycle, but DMA and
    SBUF halve -> faster startup, FWL weight loads, smaller output tail).
    PSUM accumulation stays fp32.
  - Softmax denominator: exp tiles are summed elementwise by THREE chains
    (two on DVE, one on the otherwise-idle GPSIMD, each starting with a
    2-input add) and reduced over partitions by 3 accumulated ones-matmuls
    per (h,chunk) instead of 16 -> frees ~107k PE cycles (~45us) while
    keeping DVE under the PE rate.
  - qT/kT/avT/v tiles double-buffered across batches so batch 1's
    projections overlap batch 0's attention (kills the mid-kernel stall).
  - Batch 1's projection runs QK for all chunks FIRST, then V (x re-read
    from HBM): batch 1's attention scores+exps then overlap its own V pass,
    hiding part of the ~90us of ACT exp work (the softest engine bound).
  - PSUM (8 banks): proj 2 (QK+V groups + batch-1 O-proj), scores 3
    (+ batch-0 denominators), AV 2, batch-0 O-proj 1 (trickles under
    attention); batch-1 denominators use the freed O bank; the last chunk's
    O-proj also borrows the freed score banks to shorten the tail.
  - Output projection emitted per chunk for both batches; y written bf16 in
    half-row DMAs so the final drain is ~1MB; first x/w tile pairs go out
    on the scalar/gpsimd DMA queues to beat the sync-queue spin-up.
"""

import functools
from contextlib import ExitStack

import numpy as np

D_MODEL = 2048
NUM_HEADS = 16
DK = 128
B = 2
T = 2048
BT = B * T
N_CORES = 8
H_LOC = NUM_HEADS // N_CORES  # 2 heads per core
D_LOC = H_LOC * DK  # 256
C_TILES = D_MODEL // 128  # 16
TQ = 512  # tq chunk width (one PSUM bank in fp32)
NCH = T // TQ  # 4 chunks per batch
TK_TILES = T // 128  # 16


def _body(ctx, tc, xT, wqkv, bqk, wo, y):
    import concourse.bass as bass  # noqa: F401
    from concourse import mybir

    nc = tc.nc
    f32 = mybir.dt.float32
    bf16 = mybir.dt.bfloat16
    Exp = mybir.ActivationFunctionType.Exp
    inv_sqrt_dk = 1.0 / float(np.sqrt(DK))

    # ---------------- resident tensors ----------------
    # Interleave the first x-chunk's tile loads with the weight loads so the
    # first projection matmuls can start after ~2 DMAs instead of after the
    # whole weight preload.
    wpool = ctx.enter_context(tc.tile_pool(name="wpool", bufs=1))
    x_pool = ctx.enter_context(tc.tile_pool(name="x_pool", bufs=48))

    w_tiles = []
    xt_pre = []
    for i in range(C_TILES):
        # first pairs go on otherwise-idle engine DMA queues so tile 0 lands
        # in ~1us instead of waiting behind the whole wave on sync's queues
        eng = {1: nc.scalar, 2: nc.gpsimd}.get(i, nc.sync)
        xti = x_pool.tile([128, TQ], bf16, tag="xt", name=f"xtpre{i}")
        eng.dma_start(out=xti, in_=xT[i * 128 : (i + 1) * 128, 0:TQ])
        xt_pre.append(xti)
        wt = wpool.tile([128, 3 * D_LOC], bf16, tag=f"w{i}", name=f"w{i}")
        eng.dma_start(out=wt, in_=wqkv[i * 128 : (i + 1) * 128, :])
        w_tiles.append(wt)
    bqk_sb = wpool.tile([128, 4], f32, tag="bqk", name="bqk")
    nc.sync.dma_start(out=bqk_sb, in_=bqk[:, :])

    wo_tiles = []
    for d in range(H_LOC):
        wot = wpool.tile([128, D_MODEL], bf16, tag=f"wo{d}", name=f"wo{d}")
        nc.sync.dma_start(out=wot, in_=wo[d * 128 : (d + 1) * 128, :])
        wo_tiles.append(wot)

    ones = wpool.tile([128, 128], bf16, tag="ones", name="ones")
    nc.vector.memset(ones, 1.0)

    # ---------------- pools ----------------
    qkv_pool = ctx.enter_context(tc.tile_pool(name="qkv_pool", bufs=2))
    av_pool = ctx.enter_context(tc.tile_pool(name="av_pool", bufs=2))
    es_pool = ctx.enter_context(tc.tile_pool(name="es_pool", bufs=8))
    acc_pool = ctx.enter_context(tc.tile_pool(name="acc_pool", bufs=6))
    rc_pool = ctx.enter_context(tc.tile_pool(name="rc_pool", bufs=2))
    y_pool = ctx.enter_context(tc.tile_pool(name="y_pool", bufs=3))

    # PSUM: 8 banks total.
    #   ps_p  x2: QK-proj + V-proj groups (+ batch-1's O-proj psums — proj done)
    #   ps_s  x3: score tiles + the per-unit denominator psum
    #   ps_av x2: AV accumulators (double-buffered across units)
    #   ps_o  x1: batch-0's O-proj psums (trickle, absorbed by attention gaps)
    ps_p = ctx.enter_context(tc.tile_pool(name="ps_p", bufs=2, space="PSUM"))
    ps_s = ctx.enter_context(tc.tile_pool(name="ps_s", bufs=3, space="PSUM"))
    ps_av = ctx.enter_context(tc.tile_pool(name="ps_av", bufs=2, space="PSUM"))
    ps_o = ctx.enter_context(tc.tile_pool(name="ps_o", bufs=1, space="PSUM"))



    for b in range(B):
        # ---------------- phase P: QKV projections ----------------
        qT = [
            qkv_pool.tile([128, T], bf16, tag=f"qT{d}", name=f"qT{d}_{b}")
            for d in range(H_LOC)
        ]
        kT = [
            qkv_pool.tile([128, T], bf16, tag=f"kT{d}", name=f"kT{d}_{b}")
            for d in range(H_LOC)
        ]
        v_t = [
            qkv_pool.tile([128, D_LOC], bf16, tag=f"v{t}", name=f"v{t}_{b}")
            for t in range(TK_TILES)
        ]

        def load_x_chunk(ch, pass_name):
            t0 = b * T + ch * TQ
            xt = []
            for i in range(C_TILES):
                xti = x_pool.tile(
                    [128, TQ], bf16, tag="xt", name=f"xt{pass_name}{b}_{ch}_{i}"
                )
                nc.sync.dma_start(
                    out=xti, in_=xT[i * 128 : (i + 1) * 128, t0 : t0 + TQ]
                )
                xt.append(xti)
            return xt

        def emit_qk(ch, xt):
            # QT / KT: j -> (qT, kT)[j // 2][j % 2]
            for j, dest in enumerate((qT[0], qT[1], kT[0], kT[1])):
                # chunk 0 only: borrow the (idle until attention) o/av banks so
                # four projection groups can overlap the initial x/w DMAs.
                if b == 0 and ch == 0 and j == 2:
                    ps = ps_o.tile([128, TQ], f32, tag="o", name=f"psqk02_{j}")
                elif b == 0 and ch == 0 and j == 3:
                    ps = ps_av.tile([128, TQ], f32, tag="av", name=f"psqk03_{j}")
                else:
                    ps = ps_p.tile(
                        [128, TQ], f32, tag="p", name=f"psqk{b}_{ch}_{j}"
                    )
                for i in range(C_TILES):
                    nc.tensor.matmul(
                        ps,
                        w_tiles[i][:, j * 128 : (j + 1) * 128],
                        xt[i],
                        start=(i == 0),
                        stop=(i == C_TILES - 1),
                    )
                # PSUM -> SBUF with per-partition bias add
                nc.vector.tensor_scalar_add(
                    dest[:, ch * TQ : (ch + 1) * TQ], ps, bqk_sb[:, j : j + 1]
                )

        def emit_v(ch, xt):
            # V: natural [t, d] layout (bv folded host-side as bv @ Wo)
            for ts in range(TQ // 128):
                t_idx = ch * (TQ // 128) + ts
                ps = ps_p.tile([128, TQ], f32, tag="p", name=f"psv{b}_{t_idx}")
                psv = ps[:, :D_LOC]
                for i in range(C_TILES):
                    nc.tensor.matmul(
                        psv,
                        xt[i][:, ts * 128 : (ts + 1) * 128],
                        w_tiles[i][:, 2 * D_LOC : 3 * D_LOC],
                        start=(i == 0),
                        stop=(i == C_TILES - 1),
                    )
                nc.vector.tensor_copy(v_t[t_idx], psv)

        if b == 0:
            # interleaved QK+V per chunk; batch-1's projection fills this
            # batch's attention gaps later.
            for ch in range(NCH):
                xt = xt_pre if ch == 0 else load_x_chunk(ch, "qv")
                emit_qk(ch, xt)
                emit_v(ch, xt)
        else:
            # QK for all chunks first, then V (x re-read from HBM): the last
            # batch's attention scores+exps can then overlap its own V pass,
            # hiding part of the ~90us of ACT exp work behind V matmuls.
            for ch in range(NCH):
                emit_qk(ch, load_x_chunk(ch, "q"))
            for ch in range(NCH):
                emit_v(ch, load_x_chunk(ch, "v"))

        # ---------------- phase A: attention + output projection ----------
        avT = [
            av_pool.tile([128, T], bf16, tag=f"avT{d}", name=f"avT{d}_{b}")
            for d in range(H_LOC)
        ]

        # denominator chains: c on GPSIMD (SBUF-only; idle engine) gets early
        # tiles so its slower ops finish before the pdn matmuls; a,b on DVE.
        # Each chain starts with a 2-input add (no init copy needed).
        CHAINS = {  # tk -> (chain index, peer tk for the starting add)
            0: (2, None), 2: (2, 0), 4: (2, None), 6: (2, None),
            8: (2, None), 10: (2, None),
            1: (0, None), 5: (0, 1), 9: (0, None), 12: (0, None), 14: (0, None),
            3: (1, None), 7: (1, 3), 11: (1, None), 13: (1, None), 15: (1, None),
        }

        for ch in range(NCH):
            for h in range(H_LOC):
                pav = ps_av.tile([128, TQ], f32, tag="av", name=f"pav{b}_{h}_{ch}")
                accs = [
                    acc_pool.tile(
                        [128, TQ], bf16, tag="acc", name=f"acc{cn}{b}_{h}_{ch}"
                    )
                    for cn in "abc"
                ]
                es_tiles = {}
                for tk in range(TK_TILES):
                    pss = ps_s.tile(
                        [128, TQ], f32, tag="s", name=f"pss{b}_{h}_{ch}_{tk}"
                    )
                    nc.tensor.matmul(
                        pss,
                        kT[h][:, tk * 128 : (tk + 1) * 128],
                        qT[h][:, ch * TQ : (ch + 1) * TQ],
                        start=True,
                        stop=True,
                    )
                    es = es_pool.tile(
                        [128, TQ], bf16, tag="es", name=f"es{b}_{h}_{ch}_{tk}"
                    )
                    nc.scalar.activation(es, pss, Exp, scale=inv_sqrt_dk)
                    es_tiles[tk] = es
                    ci, peer = CHAINS[tk]
                    eng = nc.gpsimd if ci == 2 else nc.vector
                    if peer is not None:
                        eng.tensor_add(accs[ci], es_tiles[peer], es)
                    elif peer is None and tk not in (0, 1, 3):
                        eng.tensor_add(accs[ci], accs[ci], es)
                    nc.tensor.matmul(
                        pav,
                        v_t[tk][:, h * 128 : (h + 1) * 128],
                        es,
                        start=(tk == 0),
                        stop=(tk == TK_TILES - 1),
                    )
                # partition-sum of the three chains in one PSUM accumulation.
                # b=1: the o-bank is free (no b1 O-proj there) — using it keeps
                # the next unit's score psums off the recip's rotation chain.
                if b == 1:
                    pdn = ps_o.tile([128, TQ], f32, tag="o", name=f"pdn{b}_{h}_{ch}")
                else:
                    pdn = ps_s.tile([128, TQ], f32, tag="s", name=f"pdn{b}_{h}_{ch}")
                for ci in range(3):
                    nc.tensor.matmul(
                        pdn, ones, accs[ci], start=(ci == 0), stop=(ci == 2)
                    )
                rc = rc_pool.tile([128, TQ], f32, tag="rc", name=f"rc{b}_{h}_{ch}")
                nc.vector.reciprocal_approx_fast(out=rc, in_=pdn)
                nc.vector.tensor_mul(avT[h][:, ch * TQ : (ch + 1) * TQ], pav, rc)

            # output projection for the t-tiles whose avT chunk just finished.
            # b=0: the single o-bank trickles under attention/proj-b1 matmuls;
            # b=1: proj is done, so the p-pool's 2 banks pipeline; the last
            # chunk also borrows the (finished) s-pool to shorten the tail.
            last_chunk = b == B - 1 and ch == NCH - 1
            for t in range(ch * (TQ // 128), (ch + 1) * (TQ // 128)):
                row0 = b * T + t * 128
                ystage = y_pool.tile(
                    [128, D_MODEL], bf16, tag="y", name=f"ys{b}_{t}"
                )
                for nch in range(NCH):
                    if b == 0:
                        pso = ps_o.tile(
                            [128, TQ], f32, tag="o", name=f"pso{b}_{t}_{nch}"
                        )
                    elif last_chunk and nch % 2 == 1:
                        pso = ps_s.tile(
                            [128, TQ], f32, tag="s", name=f"pso{b}_{t}_{nch}"
                        )
                    else:
                        pso = ps_p.tile(
                            [128, TQ], f32, tag="p", name=f"pso{b}_{t}_{nch}"
                        )
                    for d in range(H_LOC):
                        nc.tensor.matmul(
                            pso,
                            avT[d][:, t * 128 : (t + 1) * 128],
                            wo_tiles[d][:, nch * TQ : (nch + 1) * TQ],
                            start=(d == 0),
                            stop=(d == H_LOC - 1),
                        )
                    # b=0: one copy in four goes to the scalar engine; b=1 the
                    # scalar engine is exp-bound, so everything stays on DVE.
                    dst = ystage[:, nch * TQ : (nch + 1) * TQ]
                    if nch == 3 and b == 0:
                        nc.scalar.copy(dst, pso)
                    else:
                        nc.vector.tensor_copy(dst, pso)
                    # write out each half as soon as its two copies land, so
                    # the final drain is ~1MB, not the whole chunk
                    dma_eng = nc.sync
                    if nch == 1:
                        dma_eng.dma_start(
                            out=y[row0 : row0 + 128, : D_MODEL // 2],
                            in_=ystage[:, : D_MODEL // 2],
                        )
                    elif nch == 3:
                        dma_eng.dma_start(
                            out=y[row0 : row0 + 128, D_MODEL // 2 :],
                            in_=ystage[:, D_MODEL // 2 :],
                        )


@functools.cache
def _build():
    from concourse import bacc
    import concourse.tile as tile
    from concourse import mybir

    nc = bacc.Bacc(
        "TRN2",
        target_bir_lowering=False,
        debug=False,
        enable_asserts=False,
        num_devices=N_CORES,
    )
    f32 = mybir.dt.float32
    bf16 = mybir.dt.bfloat16
    xT = nc.dram_tensor("xT", [D_MODEL, BT], bf16, kind="ExternalInput").ap()
    wqkv = nc.dram_tensor(
        "wqkv", [D_MODEL, 3 * D_LOC], bf16, kind="ExternalInput"
    ).ap()
    bqk = nc.dram_tensor("bqk", [128, 4], f32, kind="ExternalInput").ap()
    wo = nc.dram_tensor("wo", [D_LOC, D_MODEL], bf16, kind="ExternalInput").ap()
    y = nc.dram_tensor("y", [BT, D_MODEL], bf16, kind="ExternalOutput").ap()

    with tile.TileContext(nc) as tc:
        with ExitStack() as ctx:
            _body(ctx, tc, xT, wqkv, bqk, wo, y)
    nc.compile()
    return nc


def _shard_inputs(x, Wq, bq, Wk, bk, Wv, bv, Wo, bo):
    """Host-side sharding: returns per-core input maps (bf16 operands)."""
    import ml_dtypes

    f = np.float32
    b16 = ml_dtypes.bfloat16
    xT = np.ascontiguousarray(
        np.asarray(x, f).reshape(BT, D_MODEL).T.astype(b16)
    )
    Wq, Wk, Wv, Wo = (np.asarray(a, f) for a in (Wq, Wk, Wv, Wo))
    bq, bk, bv = (np.asarray(a, f) for a in (bq, bk, bv))
    in_maps = []
    for c in range(N_CORES):
        sl = slice(c * D_LOC, (c + 1) * D_LOC)
        wqkv_pad = np.ascontiguousarray(
            np.concatenate([Wq[:, sl], Wk[:, sl], Wv[:, sl]], axis=1).astype(b16)
        )
        bqk_t = np.ascontiguousarray(
            np.stack(
                [
                    bq[sl][:128],
                    bq[sl][128:],
                    bk[sl][:128],
                    bk[sl][128:],
                ],
                axis=1,
            )
        )
        wo_loc = np.ascontiguousarray(Wo[sl, :].astype(b16))
        in_maps.append({"xT": xT, "wqkv": wqkv_pad, "bqk": bqk_t, "wo": wo_loc})
    return in_maps


def _run(in_maps, trace=False, **kwargs):
    from concourse.bass_utils import run_bass_kernel_spmd

    nc = _build()
    return run_bass_kernel_spmd(
        nc, in_maps, core_ids=list(range(N_CORES)), trace=trace, **kwargs
    )


def kernel(x, Wq, bq, Wk, bk, Wv, bv, Wo, bo):
    in_maps = _shard_inputs(x, Wq, bq, Wk, bk, Wv, bv, Wo, bo)
    res = _run(in_maps, trace=False)
    acc = np.zeros((BT, D_MODEL), np.float32)
    for rmap in res.results:
        acc += np.asarray(rmap["y"]).astype(np.float32)
    acc += np.asarray(bo, np.float32)[None, :]
    acc += (np.asarray(bv, np.float32) @ np.asarray(Wo, np.float32))[None, :]
    return acc.reshape(B, T, D_MODEL)


# revision 27
# speedup vs baseline: 1.0175x; 1.0065x over previous
"""MultiHeadAttention forward on 8 Trainium2 NeuronCores.

Tensor-parallel over heads: each core owns 2 of 16 heads (d_loc=256 of the
2048 QKV output columns, and the matching 256 rows of Wo). Each core
computes a full-shape partial output; the host sums the 8 partials and
adds bo (+ the folded bv @ Wo term).

Problem shape: x [2, 2048, 2048], 16 heads, d_k = 128, fp32 in/out.

Design (measured 385us vs the 530us fp32r baseline; PE busy ~91%, rel err
~5e-3 vs the 2e-2 gate):
  - All matmul operands in bf16 (PE still streams 1 row/cycle, but DMA and
    SBUF halve -> faster startup, FWL weight loads, smaller output tail).
    PSUM accumulation stays fp32.
  - Softmax denominator: exp tiles are summed elementwise by THREE chains
    (two on DVE, one on the otherwise-idle GPSIMD, each starting with a
    2-input add) and reduced over partitions by 3 accumulated ones-matmuls
    per (h,chunk) instead of 16 -> frees ~107k PE cycles (~45us) while
    keeping DVE under the PE rate.
  - qT/kT/avT/v tiles double-buffered across batches so batch 1's
    projections overlap batch 0's attention (kills the mid-kernel stall).
  - Batch 1's projection runs QK for all chunks FIRST, then V (x re-read
    from HBM): batch 1's attention scores+exps then overlap its own V pass,
    hiding part of the ~90us of ACT exp work (the softest engine bound).
  - PSUM (8 banks): proj 2 (QK+V groups + batch-1 O-proj), scores 3
    (+ batch-0 denominators), AV 2, batch-0 O-proj 1 (trickles under
    attention); batch-1 denominators use the freed O bank; the last chunk's
    O-proj also borrows the freed score banks to shorten the tail.
  - Output projection emitted per chunk for both batches; y written bf16 in
    half-row DMAs so the final drain is ~1MB; first x/w tile pairs go out
    on the scalar/gpsimd DMA queues to beat the sync-queue spin-up.
"""

import functools
from contextlib import ExitStack

import numpy as np

D_MODEL = 2048
NUM_HEADS = 16
DK = 128
B = 2
T = 2048
BT = B * T
N_CORES = 8
H_LOC = NUM_HEADS // N_CORES  # 2 heads per core
D_LOC = H_LOC * DK  # 256
C_TILES = D_MODEL // 128  # 16
TQ = 512  # tq chunk width (one PSUM bank in fp32)
NCH = T // TQ  # 4 chunks per batch
TK_TILES = T // 128  # 16


def _body(ctx, tc, xT, wqkv, bqk, wo, y):
    import concourse.bass as bass  # noqa: F401
    from concourse import mybir

    nc = tc.nc
    f32 = mybir.dt.float32
    bf16 = mybir.dt.bfloat16
    Exp = mybir.ActivationFunctionType.Exp
    inv_sqrt_dk = 1.0 / float(np.sqrt(DK))

    # ---------------- resident tensors ----------------
    # Interleave the first x-chunk's tile loads with the weight loads so the
    # first projection matmuls can start after ~2 DMAs instead of after the
    # whole weight preload.
    wpool = ctx.enter_context(tc.tile_pool(name="wpool", bufs=1))
    x_pool = ctx.enter_context(tc.tile_pool(name="x_pool", bufs=48))

    w_tiles = []
    xt_pre = []
    for i in range(C_TILES):
        # first pairs go on otherwise-idle engine DMA queues so tile 0 lands
        # in ~1us instead of waiting behind the whole wave on sync's queues
        eng = {1: nc.scalar, 2: nc.gpsimd}.get(i, nc.sync)
        xti = x_pool.tile([128, TQ], bf16, tag="xt", name=f"xtpre{i}")
        eng.dma_start(out=xti, in_=xT[i * 128 : (i + 1) * 128, 0:TQ])
        xt_pre.append(xti)
        wt = wpool.tile([128, 3 * D_LOC], bf16, tag=f"w{i}", name=f"w{i}")
        eng.dma_start(out=wt, in_=wqkv[i * 128 : (i + 1) * 128, :])
        w_tiles.append(wt)
    bqk_sb = wpool.tile([128, 4], f32, tag="bqk", name="bqk")
    nc.sync.dma_start(out=bqk_sb, in_=bqk[:, :])

    wo_tiles = []
    for d in range(H_LOC):
        wot = wpool.tile([128, D_MODEL], bf16, tag=f"wo{d}", name=f"wo{d}")
        nc.sync.dma_start(out=wot, in_=wo[d * 128 : (d + 1) * 128, :])
        wo_tiles.append(wot)

    ones = wpool.tile([128, 128], bf16, tag="ones", name="ones")
    nc.vector.memset(ones, 1.0)

    # ---------------- pools ----------------
    qkv_pool = ctx.enter_context(tc.tile_pool(name="qkv_pool", bufs=2))
    av_pool = ctx.enter_context(tc.tile_pool(name="av_pool", bufs=2))
    es_pool = ctx.enter_context(tc.tile_pool(name="es_pool", bufs=8))
    acc_pool = ctx.enter_context(tc.tile_pool(name="acc_pool", bufs=6))
    rc_pool = ctx.enter_context(tc.tile_pool(name="rc_pool", bufs=2))
    y_pool = ctx.enter_context(tc.tile_pool(name="y_pool", bufs=3))

    # PSUM: 8 banks total.
    #   ps_p  x2: QK-proj + V-proj groups (+ batch-1's O-proj psums — proj done)
    #   ps_s  x3: score tiles + the per-unit denominator psum
    #   ps_av x2: AV accumulators (double-buffered across units)
    #   ps_o  x1: batch-0's O-proj psums (trickle, absorbed by attention gaps)
    ps_p = ctx.enter_context(tc.tile_pool(name="ps_p", bufs=2, space="PSUM"))
    ps_s = ctx.enter_context(tc.tile_pool(name="ps_s", bufs=3, space="PSUM"))
    ps_av = ctx.enter_context(tc.tile_pool(name="ps_av", bufs=2, space="PSUM"))
    ps_o = ctx.enter_context(tc.tile_pool(name="ps_o", bufs=1, space="PSUM"))



    for b in range(B):
        # ---------------- phase P: QKV projections ----------------
        qT = [
            qkv_pool.tile([128, T], bf16, tag=f"qT{d}", name=f"qT{d}_{b}")
            for d in range(H_LOC)
        ]
        kT = [
            qkv_pool.tile([128, T], bf16, tag=f"kT{d}", name=f"kT{d}_{b}")
            for d in range(H_LOC)
        ]
        v_t = [
            qkv_pool.tile([128, D_LOC], bf16, tag=f"v{t}", name=f"v{t}_{b}")
            for t in range(TK_TILES)
        ]

        def load_x_chunk(ch, pass_name):
            t0 = b * T + ch * TQ
            xt = []
            for i in range(C_TILES):
                xti = x_pool.tile(
                    [128, TQ], bf16, tag="xt", name=f"xt{pass_name}{b}_{ch}_{i}"
                )
                nc.sync.dma_start(
                    out=xti, in_=xT[i * 128 : (i + 1) * 128, t0 : t0 + TQ]
                )
                xt.append(xti)
            return xt

        def emit_qk(ch, xt):
            # QT / KT: j -> (qT, kT)[j // 2][j % 2]
            for j, dest in enumerate((qT[0], qT[1], kT[0], kT[1])):
                # chunk 0 only: borrow the (idle until attention) o/av banks so
                # four projection groups can overlap the initial x/w DMAs.
                if b == 0 and ch == 0 and j == 2:
                    ps = ps_o.tile([128, TQ], f32, tag="o", name=f"psqk02_{j}")
                elif b == 0 and ch == 0 and j == 3:
                    ps = ps_av.tile([128, TQ], f32, tag="av", name=f"psqk03_{j}")
                else:
                    ps = ps_p.tile(
                        [128, TQ], f32, tag="p", name=f"psqk{b}_{ch}_{j}"
                    )
                for i in range(C_TILES):
                    nc.tensor.matmul(
                        ps,
                        w_tiles[i][:, j * 128 : (j + 1) * 128],
                        xt[i],
                        start=(i == 0),
                        stop=(i == C_TILES - 1),
                    )
                # PSUM -> SBUF with per-partition bias add
                nc.vector.tensor_scalar_add(
                    dest[:, ch * TQ : (ch + 1) * TQ], ps, bqk_sb[:, j : j + 1]
                )

        def emit_v(ch, xt):
            # V: natural [t, d] layout (bv folded host-side as bv @ Wo)
            for ts in range(TQ // 128):
                t_idx = ch * (TQ // 128) + ts
                ps = ps_p.tile([128, TQ], f32, tag="p", name=f"psv{b}_{t_idx}")
                psv = ps[:, :D_LOC]
                for i in range(C_TILES):
                    nc.tensor.matmul(
                        psv,
                        xt[i][:, ts * 128 : (ts + 1) * 128],
                        w_tiles[i][:, 2 * D_LOC : 3 * D_LOC],
                        start=(i == 0),
                        stop=(i == C_TILES - 1),
                    )
                nc.vector.tensor_copy(v_t[t_idx], psv)

        if b == 0:
            # interleaved QK+V per chunk; batch-1's projection fills this
            # batch's attention gaps later.
            for ch in range(NCH):
                xt = xt_pre if ch == 0 else load_x_chunk(ch, "qv")
                emit_qk(ch, xt)
                emit_v(ch, xt)
        else:
            # QK for all chunks first, then V (x re-read from HBM): the last
            # batch's attention scores+exps can then overlap its own V pass,
            # hiding part of the ~90us of ACT exp work behind V matmuls.
            for ch in range(NCH):
                emit_qk(ch, load_x_chunk(ch, "q"))
            for ch in range(NCH):
                emit_v(ch, load_x_chunk(ch, "v"))

        # ---------------- phase A: attention + output projection ----------
        avT = [
            av_pool.tile([128, T], bf16, tag=f"avT{d}", name=f"avT{d}_{b}")
            for d in range(H_LOC)
        ]

        # denominator chains: c on GPSIMD (SBUF-only; idle engine) gets early
        # tiles so its slower ops finish before the pdn matmuls; a,b on DVE.
        # Each chain starts with a 2-input add (no init copy needed).
        CHAINS = {  # tk -> (chain index, peer tk for the starting add)
            0: (2, None), 2: (2, 0), 4: (2, None), 6: (2, None),
            8: (2, None), 10: (2, None),
            1: (0, None), 5: (0, 1), 9: (0, None), 12: (0, None), 14: (0, None),
            3: (1, None), 7: (1, 3), 11: (1, None), 13: (1, None), 15: (1, None),
        }

        for ch in range(NCH):
            for h in range(H_LOC):
                pav = ps_av.tile([128, TQ], f32, tag="av", name=f"pav{b}_{h}_{ch}")
                accs = [
                    acc_pool.tile(
                        [128, TQ], bf16, tag="acc", name=f"acc{cn}{b}_{h}_{ch}"
                    )
                    for cn in "abc"
                ]
                es_tiles = {}
                for tk in range(TK_TILES):
                    pss = ps_s.tile(
                        [128, TQ], f32, tag="s", name=f"pss{b}_{h}_{ch}_{tk}"
                    )
                    nc.tensor.matmul(
                        pss,
                        kT[h][:, tk * 128 : (tk + 1) * 128],
                        qT[h][:, ch * TQ : (ch + 1) * TQ],
                        start=True,
                        stop=True,
                    )
                    es = es_pool.tile(
                        [128, TQ], bf16, tag="es", name=f"es{b}_{h}_{ch}_{tk}"
                    )
                    nc.scalar.activation(es, pss, Exp, scale=inv_sqrt_dk)
                    es_tiles[tk] = es
                    ci, peer = CHAINS[tk]
                    # b=1: es15 skips the chain — it feeds the denominator
                    # matmul directly, so the recip chain starts right after
                    # the last exp instead of waiting for one more DVE add.
                    if not (b == 1 and tk == 15):
                        eng = nc.gpsimd if ci == 2 else nc.vector
                        if peer is not None:
                            eng.tensor_add(accs[ci], es_tiles[peer], es)
                        elif peer is None and tk not in (0, 1, 3):
                            eng.tensor_add(accs[ci], accs[ci], es)
                    nc.tensor.matmul(
                        pav,
                        v_t[tk][:, h * 128 : (h + 1) * 128],
                        es,
                        start=(tk == 0),
                        stop=(tk == TK_TILES - 1),
                    )
                # partition-sum of the three chains in one PSUM accumulation.
                # b=1: the o-bank is free (no b1 O-proj there) — using it keeps
                # the next unit's score psums off the recip's rotation chain.
                if b == 1:
                    pdn = ps_o.tile([128, TQ], f32, tag="o", name=f"pdn{b}_{h}_{ch}")
                    srcs = accs + [es_tiles[15]]
                else:
                    pdn = ps_s.tile([128, TQ], f32, tag="s", name=f"pdn{b}_{h}_{ch}")
                    srcs = accs
                for ci, src in enumerate(srcs):
                    nc.tensor.matmul(
                        pdn, ones, src,
                        start=(ci == 0), stop=(ci == len(srcs) - 1),
                    )
                rc = rc_pool.tile([128, TQ], f32, tag="rc", name=f"rc{b}_{h}_{ch}")
                nc.vector.reciprocal_approx_fast(out=rc, in_=pdn)
                nc.vector.tensor_mul(avT[h][:, ch * TQ : (ch + 1) * TQ], pav, rc)

            # output projection for the t-tiles whose avT chunk just finished.
            # b=0: the single o-bank trickles under attention/proj-b1 matmuls;
            # b=1: proj is done, so the p-pool's 2 banks pipeline; the last
            # chunk also borrows the (finished) s-pool to shorten the tail.
            last_chunk = b == B - 1 and ch == NCH - 1
            for t in range(ch * (TQ // 128), (ch + 1) * (TQ // 128)):
                row0 = b * T + t * 128
                ystage = y_pool.tile(
                    [128, D_MODEL], bf16, tag="y", name=f"ys{b}_{t}"
                )
                for nch in range(NCH):
                    if b == 0:
                        pso = ps_o.tile(
                            [128, TQ], f32, tag="o", name=f"pso{b}_{t}_{nch}"
                        )
                    elif last_chunk and nch % 2 == 1:
                        pso = ps_s.tile(
                            [128, TQ], f32, tag="s", name=f"pso{b}_{t}_{nch}"
                        )
                    else:
                        pso = ps_p.tile(
                            [128, TQ], f32, tag="p", name=f"pso{b}_{t}_{nch}"
                        )
                    for d in range(H_LOC):
                        nc.tensor.matmul(
                            pso,
                            avT[d][:, t * 128 : (t + 1) * 128],
                            wo_tiles[d][:, nch * TQ : (nch + 1) * TQ],
                            start=(d == 0),
                            stop=(d == H_LOC - 1),
                        )
                    # b=0: one copy in four goes to the scalar engine; b=1 the
                    # scalar engine is exp-bound, so everything stays on DVE.
                    dst = ystage[:, nch * TQ : (nch + 1) * TQ]
                    if (nch == 3 and b == 0) or (last_chunk and nch % 2 == 1):
                        # last chunk: exps are done, the scalar engine is free
                        # to halve the tail's copy stream
                        nc.scalar.copy(dst, pso)
                    else:
                        nc.vector.tensor_copy(dst, pso)
                    # write out each half as soon as its two copies land, so
                    # the final drain is ~1MB, not the whole chunk
                    dma_eng = nc.sync
                    if nch == 1:
                        dma_eng.dma_start(
                            out=y[row0 : row0 + 128, : D_MODEL // 2],
                            in_=ystage[:, : D_MODEL // 2],
                        )
                    elif nch == 3:
                        dma_eng.dma_start(
                            out=y[row0 : row0 + 128, D_MODEL // 2 :],
                            in_=ystage[:, D_MODEL // 2 :],
                        )


@functools.cache
def _build():
    from concourse import bacc
    import concourse.tile as tile
    from concourse import mybir

    nc = bacc.Bacc(
        "TRN2",
        target_bir_lowering=False,
        debug=False,
        enable_asserts=False,
        num_devices=N_CORES,
    )
    f32 = mybir.dt.float32
    bf16 = mybir.dt.bfloat16
    xT = nc.dram_tensor("xT", [D_MODEL, BT], bf16, kind="ExternalInput").ap()
    wqkv = nc.dram_tensor(
        "wqkv", [D_MODEL, 3 * D_LOC], bf16, kind="ExternalInput"
    ).ap()
    bqk = nc.dram_tensor("bqk", [128, 4], f32, kind="ExternalInput").ap()
    wo = nc.dram_tensor("wo", [D_LOC, D_MODEL], bf16, kind="ExternalInput").ap()
    y = nc.dram_tensor("y", [BT, D_MODEL], bf16, kind="ExternalOutput").ap()

    with tile.TileContext(nc) as tc:
        with ExitStack() as ctx:
            _body(ctx, tc, xT, wqkv, bqk, wo, y)
    nc.compile()
    return nc


def _shard_inputs(x, Wq, bq, Wk, bk, Wv, bv, Wo, bo):
    """Host-side sharding: returns per-core input maps (bf16 operands)."""
    import ml_dtypes

    f = np.float32
    b16 = ml_dtypes.bfloat16
    xT = np.ascontiguousarray(
        np.asarray(x, f).reshape(BT, D_MODEL).T.astype(b16)
    )
    Wq, Wk, Wv, Wo = (np.asarray(a, f) for a in (Wq, Wk, Wv, Wo))
    bq, bk, bv = (np.asarray(a, f) for a in (bq, bk, bv))
    in_maps = []
    for c in range(N_CORES):
        sl = slice(c * D_LOC, (c + 1) * D_LOC)
        wqkv_pad = np.ascontiguousarray(
            np.concatenate([Wq[:, sl], Wk[:, sl], Wv[:, sl]], axis=1).astype(b16)
        )
        bqk_t = np.ascontiguousarray(
            np.stack(
                [
                    bq[sl][:128],
                    bq[sl][128:],
                    bk[sl][:128],
                    bk[sl][128:],
                ],
                axis=1,
            )
        )
        wo_loc = np.ascontiguousarray(Wo[sl, :].astype(b16))
        in_maps.append({"xT": xT, "wqkv": wqkv_pad, "bqk": bqk_t, "wo": wo_loc})
    return in_maps


def _run(in_maps, trace=False, **kwargs):
    from concourse.bass_utils import run_bass_kernel_spmd

    nc = _build()
    return run_bass_kernel_spmd(
        nc, in_maps, core_ids=list(range(N_CORES)), trace=trace, **kwargs
    )


def kernel(x, Wq, bq, Wk, bk, Wv, bv, Wo, bo):
    in_maps = _shard_inputs(x, Wq, bq, Wk, bk, Wv, bv, Wo, bo)
    res = _run(in_maps, trace=False)
    acc = np.zeros((BT, D_MODEL), np.float32)
    for rmap in res.results:
        acc += np.asarray(rmap["y"]).astype(np.float32)
    acc += np.asarray(bo, np.float32)[None, :]
    acc += (np.asarray(bv, np.float32) @ np.asarray(Wo, np.float32))[None, :]
    return acc.reshape(B, T, D_MODEL)
